# revision 44
# baseline (speedup 1.0000x reference)
"""Additive attention (d2l-style) on 8 Trainium2 NeuronCores — low-rank scores.

reference math per batch b (B=8, Q=256, K=512, D=256, H=128):
    scores[q, k] = sum_h W_v[h] * tanh(qf[h,q] + kf[h,k])
    attn = softmax_k(scores), masked to k < valid_length[b]
    out  = attn @ value

Numerics (unchanged from the 23.2us baseline): a Gaussian-weighted
separable expansion tanh(x+y) ~= sum_t f_t(x) g_t(y) with R=6 terms
(t0,t1 bf16; t2..5 fp8-e4m3 contracted pairwise with DoubleRow matmuls)
turns the score computation into PE matmuls with contraction R*H;
exp on ACT; E^T @ [V | 1] gives numerator|denominator which the host
divides during unshard.  Host prepares the O((Q+K)*H) feature maps.

Schedule findings baked in (from perfetto analysis of 7 variants; best
measured 19.6us vs the 23.2us session baseline on the same box):
- Semaphore updates go through a serial unit (~73ns each) and every DMA
  completion costs 16 increments (one per DMA engine), so a consumer's
  release lags its data by >=1.2us and each extra DMA delays every
  later one.  The input is therefore exactly 3 DMAs, all serial on
  the Sync HWDGE queue in compute need order: [pk0: all of chunk 0 +
  U][g123: chunks 1-3][pkv: V].  One queue makes every release
  deterministic — with V on the other queue, DMA-engine arbitration
  could push its packets into g123's window and slip g123's release
  past chunk-1's need (and V's own release past EV-0's).  The final
  trace shows zero event-waits >200ns anywhere in the real stream.
- The PE dispatches 256-free bf16 matmuls at 213ns (1.2GHz) and steps
  to 107ns (2.4GHz) only after ~3-5us of *continuous* busy; an idle
  gap resets the ramp.  A junk-matmul train sized to the DMA+semaphore
  pipeline depth (~3.6us) runs from program entry and hands off to the
  first real matmul with no gap, so the real compute runs mostly at
  full clock (DoubleRow fp8 matmuls observed at 53-67ns).
- All four score chunks run back to back (their stop events pace the
  serial ACT exp chain, which is the mid-phase critical path), with
  the EV block at the end as each exp completes; junk pads fill the
  two predictable waits (g123 release, exp2) to protect the ramp.
- Each score chunk gets its own PSUM bank (4 sc + 2 out + 1 junk of 8)
  so no matmul waits WAR on exp reading a recycled bank.
- Output halves cast on different engines (ACT copy for h0 right after
  the last exp; DVE for h1) and ship concurrently on both HWDGE queues.
- Run-to-run spread comes from the chip, not the schedule: the box is
  bimodal (down-clocks ~20% for minutes at a time) and the MID->FULL
  ramp step lands with ~1-2us jitter; the junk sizing self-tracks both
  because the warmup matmuls and the DMA machinery scale together.
"""

import sys
from contextlib import ExitStack

if "/opt/trn_rl_repo" not in sys.path:
    sys.path.insert(0, "/opt/trn_rl_repo")

import numpy as np

B, Q, K, D, H, V = 8, 256, 512, 256, 128, 256
NCORES = 8
R = 6          # separable rank of tanh(x+y)
NCH = K // 128  # key chunks per core (uniform; pads are data, not code)
A_LIM = 8.0
NGRID = 1601
N8 = R - 2     # fp8 components
VW = V + 1

# schedule tuning knobs (tuned against perfetto traces)
NJUNK = 14     # free-256 warmup matmuls from program entry (~213ns each)
NJUNK_TAIL = 4  # free-128 warmup matmuls (~107ns each) for fine overrun
JPAD_SC1 = 3   # junk before chunk-1 scores (g123 release lag)
JPAD_EV0 = 0   # pkv release is deterministic now; pads here were hoisted anyway
JPAD_EV2 = 1   # absorbs the ~0.5us EV2-on-exp2 wait below the ramp-reset zone

_NC_CACHE = None
_BASIS = None
_LAST_RESULTS = None


def _basis():
    """Gaussian-weighted separable expansion tanh(x+y) ~= sum_t f_t(x)g_t(y)."""
    global _BASIS
    if _BASIS is None:
        x = np.linspace(-A_LIM, A_LIM, NGRID)
        w = np.exp(-0.5 * x**2) / np.sqrt(2 * np.pi) + 1e-4
        sw = np.sqrt(w)
        Aw = sw[:, None] * np.tanh(x[:, None] + x[None, :]) * sw[None, :]
        lam, phi = np.linalg.eigh(Aw)  # symmetric kernel
        idx = np.argsort(-np.abs(lam))[:R]
        lam, phi = lam[idx], phi[:, idx]
        ftab = phi * np.sqrt(np.abs(lam))[None, :] / sw[:, None]
        gtab = ftab * np.sign(lam)[None, :]
        _BASIS = (x, ftab, gtab)
    return _BASIS


def _build():
    from concourse import bacc, mybir, tile

    f32 = mybir.dt.float32
    bf16 = mybir.dt.bfloat16
    f8 = mybir.dt.float8e4

    nc = bacc.Bacc(
        "TRN2",
        target_bir_lowering=False,
        debug=False,
        enable_asserts=False,
        num_devices=NCORES,
    )

    # pack layout (bf16 carrier elements; fp8 sections bitcast-viewed):
    #   pk0 = U_bf | G0_bf | U_8 | G0_8     (all chunk-0 needs + U)
    #   gc  = Gc_bf | Gc_8                  (c = 1..3)
    #   pkv = V0 | V1 | V2 | V3             (ones column appended to each)
    UBW, GBW = 2 * Q, 2 * 128
    U8W, G8W = N8 * Q // 2, N8 * 128 // 2  # fp8 widths in bf16 elems
    PK0W = UBW + GBW + U8W + G8W
    GW = GBW + G8W
    pk0_d = nc.dram_tensor("pk0", [128, PK0W], bf16, kind="ExternalInput")
    g123_d = nc.dram_tensor("g123", [128, (NCH - 1) * GW], bf16,
                            kind="ExternalInput")
    pkv_d = nc.dram_tensor("pkv", [128, NCH * VW], bf16, kind="ExternalInput")
    out_d = nc.dram_tensor("out", [128, 2 * VW], bf16, kind="ExternalOutput")

    Exp = mybir.ActivationFunctionType.Exp
    Copy = mybir.ActivationFunctionType.Copy
    DR = mybir.MatmulPerfMode.DoubleRow

    with tile.TileContext(nc) as tc, ExitStack() as ctx:
        sb = ctx.enter_context(tc.tile_pool(name="sb", bufs=1))
        ps = ctx.enter_context(tc.tile_pool(name="ps", bufs=1, space="PSUM"))

        pk0_t = sb.tile([128, PK0W], bf16, tag="pk0", name="pk0")
        g123_t = sb.tile([128, (NCH - 1) * GW], bf16, tag="g123", name="g123")
        pkv_t = sb.tile([128, NCH * VW], bf16, tag="pkv", name="pkv")

        # Input DMA plan (3 DMAs total — each DMA's completion costs 16
        # serial ~73ns semaphore updates, processed in packet-arrival order,
        # so fewer DMAs release everything sooner): all three ride the Sync
        # HWDGE queue serially in compute need order, which makes every
        # release deterministic — on a second queue, DMA-engine arbitration
        # could push V's packets into g123's window and slip g123's release
        # past chunk-1's need (and V's own release past EV-0's).
        nc.sync.dma_start(pk0_t[:, :], pk0_d[:, :])
        nc.sync.dma_start(g123_t[:, :], g123_d[:, :])
        nc.sync.dma_start(pkv_t[:, :], pkv_d[:, :])

        # junk-tile memset on the otherwise-idle Pool engine; exp table
        # preload off the critical path
        junk = sb.tile([128, 256], bf16, tag="junk")
        nc.gpsimd.memset(junk[:, :], 0.0)
        warm = sb.tile([1, 1], f32, tag="warm")
        nc.gpsimd.memset(warm[:, :], 0.0)
        nc.scalar.activation(warm[:, :], warm[:, :], Exp)

        # PE p-state warmup covering the whole DMA lead-in (see module doc)
        jp = ps.tile([128, 256], f32, tag="jp")
        for i in range(NJUNK):
            nc.tensor.matmul(
                jp[:, :], junk[:, :128], junk[:, :], start=(i == 0),
                stop=False,
            )
        for i in range(NJUNK_TAIL):
            nc.tensor.matmul(
                jp[:, :128], junk[:, :128], junk[:, :128], start=False,
                stop=(i == NJUNK_TAIL - 1),
            )

        def jpad(n):
            for _ in range(n):
                nc.tensor.matmul(jp[:, :128], junk[:, :128], junk[:, :128],
                                 start=True, stop=True)

        # fp8 sections, bitcast-viewed ([128, n] bf16 -> [128, 2n] fp8)
        f8v0 = pk0_t[:, UBW + GBW :].bitcast(f8)

        def u_bf(t):
            return pk0_t[:, t * Q : (t + 1) * Q]

        def g0_bf(t):
            return pk0_t[:, UBW + t * 128 : UBW + (t + 1) * 128]

        def u_8pair(i):
            sl = f8v0[:, 2 * i * Q : 2 * (i + 1) * Q]
            return sl.rearrange("p (two f) -> p two f", two=2)

        def g0_8pair(i):
            off = 2 * U8W
            sl = f8v0[:, off + i * 256 : off + (i + 1) * 256]
            return sl.rearrange("p (two f) -> p two f", two=2)

        def g_bf(c, t):  # c >= 1
            off = (c - 1) * GW
            return g123_t[:, off + t * 128 : off + (t + 1) * 128]

        def g_8pair(c, i):  # c >= 1
            off = (c - 1) * GW + GBW
            view = g123_t[:, off : off + G8W].bitcast(f8)
            sl = view[:, i * 256 : (i + 1) * 256]
            return sl.rearrange("p (two f) -> p two f", two=2)

        def v_sl(c):
            return pkv_t[:, c * VW : (c + 1) * VW]

        o_tiles = [
            ps.tile([128, VW], f32, tag=f"o{h2}", name=f"o{h2}")
            for h2 in range(2)
        ]
        sc_tiles = [
            ps.tile([128, Q], f32, tag=f"sc{c}", name=f"sc{c}")
            for c in range(NCH)
        ]
        et_tiles = [
            sb.tile([128, Q], bf16, tag=f"et{c}", name=f"et{c}")
            for c in range(NCH)
        ]

        def emit_scores(c, split_exp=False):
            sc = sc_tiles[c]
            for t in range(2):
                gb = g0_bf(t) if c == 0 else g_bf(c, t)
                nc.tensor.matmul(
                    sc[:, :], gb, u_bf(t), start=(t == 0), stop=False
                )
            for i in range(N8 // 2):
                g8 = g0_8pair(i) if c == 0 else g_8pair(c, i)
                nc.tensor.matmul(
                    sc[:, :], g8, u_8pair(i), start=False,
                    stop=(i == N8 // 2 - 1), perf_mode=DR,
                )
            et = et_tiles[c]
            if split_exp:
                # last chunk: per-half exp so EV/cast/out of h0 fire earlier
                nc.scalar.activation(et[:, :128], sc[:, :128], Exp)
                nc.scalar.activation(et[:, 128:], sc[:, 128:], Exp)
            else:
                nc.scalar.activation(et[:, :], sc[:, :], Exp)

        def emit_ev(c):
            et = et_tiles[c]
            for h2 in range(2):
                nc.tensor.matmul(
                    o_tiles[h2][:, :],
                    et[:, h2 * 128 : (h2 + 1) * 128],
                    v_sl(c),
                    start=(c == 0),
                    stop=(c == NCH - 1),
                )

        # PE: all four score chunks back to back (stops pace the exp chain,
        # so earlier stops shorten the critical path), then the EV block as
        # each exp completes.  ACT: per-chunk exp, last chunk per-half.
        emit_scores(0)
        jpad(JPAD_SC1)
        emit_scores(1)
        emit_scores(2)
        emit_scores(3, split_exp=True)
        jpad(JPAD_EV0)
        emit_ev(0)
        emit_ev(1)
        jpad(JPAD_EV2)
        emit_ev(2)
        emit_ev(3)

        # ship raw numerator|denominator; the host divides during unshard.
        # Halves cast on different engines (ACT copy for h0 — it finishes
        # first and ACT is idle after the last exp; DVE for h1) and ship on
        # both HWDGE queues so the two output paths run in parallel.
        osb = sb.tile([128, 2 * VW], bf16, tag="osb")
        nc.scalar.activation(osb[:, :VW], o_tiles[0][:, :], Copy)
        nc.scalar.dma_start(out_d[:, :VW], osb[:, :VW])
        nc.vector.tensor_copy(osb[:, VW:], o_tiles[1][:, :])
        nc.sync.dma_start(out_d[:, VW:], osb[:, VW:])

    nc.compile()
    return nc


def _feat(tab, x, pts):
    out = np.empty(pts.shape + (R,), dtype=np.float32)
    for t in range(R):
        out[..., t] = np.interp(pts, x, tab[:, t])
    return out


def _prep_in_maps(queries, key, value, W_k, W_q, W_v, Ls):
    import ml_dtypes

    bf16 = ml_dtypes.bfloat16
    f8 = ml_dtypes.float8_e4m3fn
    x, ftab, gtab = _basis()
    wv = W_v[0].astype(np.float32)

    # host projections (tiny, <1% of FLOPs — same as baseline)
    qf = np.einsum("hd,bqd->bqh", W_q, queries, optimize=True)
    kf = np.einsum("hd,bkd->bkh", W_k, key, optimize=True)

    def as_bf(a8):  # view fp8 bytes as bf16 carrier elements
        return a8.view(np.uint8).reshape(H, -1, 2).view(np.uint16).reshape(
            H, -1
        ).view(bf16)

    in_maps = []
    for b in range(B):
        L = int(Ls[b])
        # U[h, t*Q + q] = wv[h] * f_t(qf[b,q,h])
        fq = _feat(ftab, x, qf[b])                      # [Q, H, R]
        U = (fq * wv[None, :, None]).transpose(1, 2, 0)  # [H, R, Q]
        U = np.ascontiguousarray(U.reshape(H, R * Q))
        U_bf = U[:, : 2 * Q].astype(bf16)
        U_8 = U[:, 2 * Q :].astype(f8)

        # G[c, h, t*128 + j] = g_t(kf[b, c*128+j, h]), zero for k >= L;
        # pad chunks duplicate chunk 0 (finite scores under exp, V there is 0)
        gk = _feat(gtab, x, kf[b])                      # [K, H, R]
        gk[L:] = 0.0
        G = gk.transpose(1, 2, 0).reshape(H, R, NCH, 128)
        G = np.ascontiguousarray(G.transpose(2, 0, 1, 3)).reshape(
            NCH, H, R * 128
        )
        nreal = max(1, -(-L // 128))
        G[nreal:] = G[0]
        G_bf = G[:, :, : 2 * 128].astype(bf16)
        G_8 = G[:, :, 2 * 128 :].astype(f8)

        # V chunks with ones column; rows >= L zeroed
        Vv = np.zeros((K, VW), dtype=np.float32)
        Vv[:L, :V] = value[b, :L]
        Vv[:L, V] = 1.0
        Vv = Vv.reshape(NCH, 128, VW).astype(bf16)

        m = {
            "pk0": np.concatenate(
                [U_bf, G_bf[0], as_bf(U_8), as_bf(G_8[0])], axis=1
            ),
            "g123": np.concatenate(
                sum(([G_bf[c], as_bf(G_8[c])] for c in range(1, NCH)), []),
                axis=1,
            ),
            "pkv": np.ascontiguousarray(
                Vv.transpose(1, 0, 2).reshape(128, NCH * VW)
            ),
        }
        in_maps.append(m)
    return in_maps


def kernel(queries, key, value, W_k, W_q, W_v, valid_length):
    global _NC_CACHE, _LAST_RESULTS
    queries = np.asarray(queries, dtype=np.float32)
    key = np.asarray(key, dtype=np.float32)
    value = np.asarray(value, dtype=np.float32)
    W_k = np.asarray(W_k, dtype=np.float32)
    W_q = np.asarray(W_q, dtype=np.float32)
    W_v = np.asarray(W_v, dtype=np.float32)
    Ls = tuple(int(x) for x in np.asarray(valid_length).reshape(-1))
    assert len(Ls) == B and all(1 <= L <= K for L in Ls)

    if _NC_CACHE is None:
        _NC_CACHE = _build()
    nc = _NC_CACHE

    in_maps = _prep_in_maps(queries, key, value, W_k, W_q, W_v, Ls)

    from concourse.bass_utils import run_bass_kernel_spmd

    res = run_bass_kernel_spmd(nc, in_maps, core_ids=list(range(NCORES)))
    _LAST_RESULTS = res

    out = np.empty((B, Q, V), dtype=np.float32)
    for b in range(NCORES):
        raw = res.results[b]["out"].astype(np.float32).reshape(128, 2, VW)
        raw = raw.transpose(1, 0, 2).reshape(Q, VW)
        out[b] = raw[:, :V] / raw[:, V : V + 1]
    return out


# revision 45
# speedup vs baseline: 1.2220x; 1.2220x over previous
"""Additive attention (d2l-style) on 8 Trainium2 NeuronCores — low-rank scores.

reference math per batch b (B=8, Q=256, K=512, D=256, H=128):
    scores[q, k] = sum_h W_v[h] * tanh(qf[h,q] + kf[h,k])
    attn = softmax_k(scores), masked to k < valid_length[b]
    out  = attn @ value

Numerics (unchanged from the 23.2us baseline): a Gaussian-weighted
separable expansion tanh(x+y) ~= sum_t f_t(x) g_t(y) with R=6 terms
(t0,t1 bf16; t2..5 fp8-e4m3 contracted pairwise with DoubleRow matmuls)
turns the score computation into PE matmuls with contraction R*H;
exp on ACT; E^T @ [V | 1] gives numerator|denominator which the host
divides during unshard.  Host prepares the O((Q+K)*H) feature maps.

Schedule findings baked in (from perfetto analysis of 7 variants; best
measured 19.6us vs the 23.2us session baseline on the same box):
- Semaphore updates go through a serial unit (~73ns each) and every DMA
  completion costs 16 increments (one per DMA engine), so a consumer's
  release lags its data by >=1.2us and each extra DMA delays every
  later one.  The input is therefore exactly 3 DMAs, all serial on
  the Sync HWDGE queue in compute need order: [pk0: all of chunk 0 +
  U][g123: chunks 1-3][pkv: V].  One queue makes every release
  deterministic — with V on the other queue, DMA-engine arbitration
  could push its packets into g123's window and slip g123's release
  past chunk-1's need (and V's own release past EV-0's).  The final
  trace shows zero event-waits >200ns anywhere in the real stream.
- The PE dispatches 256-free bf16 matmuls at 213ns (1.2GHz) and steps
  to 107ns (2.4GHz) only after ~3-5us of *continuous* busy; an idle
  gap resets the ramp.  A junk-matmul train sized to the DMA+semaphore
  pipeline depth (~3.6us) runs from program entry and hands off to the
  first real matmul with no gap, so the real compute runs mostly at
  full clock (DoubleRow fp8 matmuls observed at 53-67ns).
- All four score chunks run back to back (their stop events pace the
  serial ACT exp chain, which is the mid-phase critical path), with
  the EV block at the end as each exp completes; junk pads fill the
  two predictable waits (g123 release, exp2) to protect the ramp.
- Each score chunk gets its own PSUM bank (4 sc + 2 out + 1 junk of 8)
  so no matmul waits WAR on exp reading a recycled bank.
- Output halves cast on different engines (ACT copy for h0 right after
  the last exp; DVE for h1) and ship concurrently on both HWDGE queues.
- Run-to-run spread comes from the chip, not the schedule: the box is
  bimodal (down-clocks ~20% for minutes at a time) and the MID->FULL
  ramp step lands with ~1-2us jitter; the junk sizing self-tracks both
  because the warmup matmuls and the DMA machinery scale together.
"""

import sys
from contextlib import ExitStack

if "/opt/trn_rl_repo" not in sys.path:
    sys.path.insert(0, "/opt/trn_rl_repo")

import numpy as np

B, Q, K, D, H, V = 8, 256, 512, 256, 128, 256
NCORES = 8
R = 6          # separable rank of tanh(x+y)
NCH = K // 128  # key chunks per core (uniform; pads are data, not code)
A_LIM = 8.0
NGRID = 1601
N8 = R - 2     # fp8 components
VW = V + 1

# schedule tuning knobs (tuned against perfetto traces)
NJUNK = 14     # free-256 warmup matmuls from program entry (~213ns each)
NJUNK_TAIL = 4  # free-128 warmup matmuls (~107ns each) for fine overrun
JPAD_SC1 = 3   # junk before chunk-1 scores (g123 release lag)
JPAD_EV0 = 0   # pkv release is deterministic now; pads here were hoisted anyway
JPAD_EV2 = 1   # absorbs the ~0.5us EV2-on-exp2 wait below the ramp-reset zone

_NC_CACHE = None
_BASIS = None
_LAST_RESULTS = None


def _basis():
    """Gaussian-weighted separable expansion tanh(x+y) ~= sum_t f_t(x)g_t(y)."""
    global _BASIS
    if _BASIS is None:
        x = np.linspace(-A_LIM, A_LIM, NGRID)
        w = np.exp(-0.5 * x**2) / np.sqrt(2 * np.pi) + 1e-4
        sw = np.sqrt(w)
        Aw = sw[:, None] * np.tanh(x[:, None] + x[None, :]) * sw[None, :]
        lam, phi = np.linalg.eigh(Aw)  # symmetric kernel
        idx = np.argsort(-np.abs(lam))[:R]
        lam, phi = lam[idx], phi[:, idx]
        ftab = phi * np.sqrt(np.abs(lam))[None, :] / sw[:, None]
        gtab = ftab * np.sign(lam)[None, :]
        _BASIS = (x, ftab, gtab)
    return _BASIS


def _build():
    from concourse import bacc, mybir, tile

    f32 = mybir.dt.float32
    bf16 = mybir.dt.bfloat16
    f8 = mybir.dt.float8e4

    nc = bacc.Bacc(
        "TRN2",
        target_bir_lowering=False,
        debug=False,
        enable_asserts=False,
        num_devices=NCORES,
    )

    # pack layout (bf16 carrier elements; fp8 sections bitcast-viewed):
    #   pk0 = U_bf | G0_bf | U_8 | G0_8     (all chunk-0 needs + U)
    #   gc  = Gc_bf | Gc_8                  (c = 1..3)
    #   pkv = V0 | V1 | V2 | V3             (ones column appended to each)
    UBW, GBW = 2 * Q, 2 * 128
    U8W, G8W = N8 * Q // 2, N8 * 128 // 2  # fp8 widths in bf16 elems
    PK0W = UBW + GBW + U8W + G8W
    GW = GBW + G8W
    pk0_d = nc.dram_tensor("pk0", [128, PK0W], bf16, kind="ExternalInput")
    g123_d = nc.dram_tensor("g123", [128, (NCH - 1) * GW], bf16,
                            kind="ExternalInput")
    pkv_d = nc.dram_tensor("pkv", [128, NCH * VW], bf16, kind="ExternalInput")
    out_d = nc.dram_tensor("out", [128, 2 * VW], bf16, kind="ExternalOutput")

    Exp = mybir.ActivationFunctionType.Exp
    Copy = mybir.ActivationFunctionType.Copy
    DR = mybir.MatmulPerfMode.DoubleRow

    with tile.TileContext(nc) as tc, ExitStack() as ctx:
        sb = ctx.enter_context(tc.tile_pool(name="sb", bufs=1))
        ps = ctx.enter_context(tc.tile_pool(name="ps", bufs=1, space="PSUM"))

        pk0_t = sb.tile([128, PK0W], bf16, tag="pk0", name="pk0")
        g123_t = sb.tile([128, (NCH - 1) * GW], bf16, tag="g123", name="g123")
        pkv_t = sb.tile([128, NCH * VW], bf16, tag="pkv", name="pkv")

        # Input DMA plan (3 DMAs total — each DMA's completion costs 16
        # serial ~73ns semaphore updates, processed in packet-arrival order,
        # so fewer DMAs release everything sooner): all three ride the Sync
        # HWDGE queue serially in compute need order, which makes every
        # release deterministic — on a second queue, DMA-engine arbitration
        # could push V's packets into g123's window and slip g123's release
        # past chunk-1's need (and V's own release past EV-0's).
        nc.sync.dma_start(pk0_t[:, :], pk0_d[:, :])
        nc.sync.dma_start(g123_t[:, :], g123_d[:, :])
        nc.sync.dma_start(pkv_t[:, :], pkv_d[:, :])

        # junk-tile memset on the otherwise-idle Pool engine; exp table
        # preload off the critical path
        junk = sb.tile([128, 256], bf16, tag="junk")
        nc.gpsimd.memset(junk[:, :], 0.0)
        warm = sb.tile([1, 1], f32, tag="warm")
        nc.gpsimd.memset(warm[:, :], 0.0)
        nc.scalar.activation(warm[:, :], warm[:, :], Exp)

        # PE p-state warmup covering the whole DMA lead-in (see module doc)
        jp = ps.tile([128, 256], f32, tag="jp")
        for i in range(NJUNK):
            nc.tensor.matmul(
                jp[:, :], junk[:, :128], junk[:, :], start=(i == 0),
                stop=False,
            )
        for i in range(NJUNK_TAIL):
            nc.tensor.matmul(
                jp[:, :128], junk[:, :128], junk[:, :128], start=False,
                stop=(i == NJUNK_TAIL - 1),
            )

        def jpad(n):
            for _ in range(n):
                nc.tensor.matmul(jp[:, :128], junk[:, :128], junk[:, :128],
                                 start=True, stop=True)

        # fp8 sections, bitcast-viewed ([128, n] bf16 -> [128, 2n] fp8)
        f8v0 = pk0_t[:, UBW + GBW :].bitcast(f8)

        def u_bf(t):
            return pk0_t[:, t * Q : (t + 1) * Q]

        def g0_bf(t):
            return pk0_t[:, UBW + t * 128 : UBW + (t + 1) * 128]

        def u_8pair(i):
            sl = f8v0[:, 2 * i * Q : 2 * (i + 1) * Q]
            return sl.rearrange("p (two f) -> p two f", two=2)

        def g0_8pair(i):
            off = 2 * U8W
            sl = f8v0[:, off + i * 256 : off + (i + 1) * 256]
            return sl.rearrange("p (two f) -> p two f", two=2)

        def g_bf(c, t):  # c >= 1
            off = (c - 1) * GW
            return g123_t[:, off + t * 128 : off + (t + 1) * 128]

        def g_8pair(c, i):  # c >= 1
            off = (c - 1) * GW + GBW
            view = g123_t[:, off : off + G8W].bitcast(f8)
            sl = view[:, i * 256 : (i + 1) * 256]
            return sl.rearrange("p (two f) -> p two f", two=2)

        def v_sl(c):
            return pkv_t[:, c * VW : (c + 1) * VW]

        o_tiles = [
            ps.tile([128, VW], f32, tag=f"o{h2}", name=f"o{h2}")
            for h2 in range(2)
        ]
        sc_tiles = [
            ps.tile([128, Q], f32, tag=f"sc{c}", name=f"sc{c}")
            for c in range(NCH)
        ]
        et_tiles = [
            sb.tile([128, Q], bf16, tag=f"et{c}", name=f"et{c}")
            for c in range(NCH)
        ]

        def emit_scores(c, split_exp=False):
            sc = sc_tiles[c]
            for t in range(2):
                gb = g0_bf(t) if c == 0 else g_bf(c, t)
                nc.tensor.matmul(
                    sc[:, :], gb, u_bf(t), start=(t == 0), stop=False
                )
            for i in range(N8 // 2):
                g8 = g0_8pair(i) if c == 0 else g_8pair(c, i)
                nc.tensor.matmul(
                    sc[:, :], g8, u_8pair(i), start=False,
                    stop=(i == N8 // 2 - 1), perf_mode=DR,
                )
            et = et_tiles[c]
            if split_exp:
                # last chunk: per-half exp so EV/cast/out of h0 fire earlier
                nc.scalar.activation(et[:, :128], sc[:, :128], Exp)
                nc.scalar.activation(et[:, 128:], sc[:, 128:], Exp)
            else:
                nc.scalar.activation(et[:, :], sc[:, :], Exp)

        def emit_ev(c):
            et = et_tiles[c]
            for h2 in range(2):
                nc.tensor.matmul(
                    o_tiles[h2][:, :],
                    et[:, h2 * 128 : (h2 + 1) * 128],
                    v_sl(c),
                    start=(c == 0),
                    stop=(c == NCH - 1),
                )

        # PE: all four score chunks back to back (stops pace the exp chain,
        # so earlier stops shorten the critical path), then the EV block as
        # each exp completes.  ACT: per-chunk exp, last chunk per-half.
        emit_scores(0)
        jpad(JPAD_SC1)
        emit_scores(1)
        emit_scores(2)
        # exp3 unsplit: the tail is ACT-bound in this geometry (the cast
        # waits for the exp chain to free ACT regardless), so one [128,256]
        # exp ends the chain ~0.26us earlier than the per-half pair
        emit_scores(3)
        jpad(JPAD_EV0)
        emit_ev(0)
        emit_ev(1)
        jpad(JPAD_EV2)
        emit_ev(2)
        emit_ev(3)

        # ship raw numerator|denominator; the host divides during unshard.
        # Halves cast on different engines (ACT copy for h0 — it finishes
        # first and ACT is idle after the last exp; DVE for h1) and ship on
        # both HWDGE queues so the two output paths run in parallel.
        osb = sb.tile([128, 2 * VW], bf16, tag="osb")
        nc.scalar.activation(osb[:, :VW], o_tiles[0][:, :], Copy)
        nc.scalar.dma_start(out_d[:, :VW], osb[:, :VW])
        nc.vector.tensor_copy(osb[:, VW:], o_tiles[1][:, :])
        nc.sync.dma_start(out_d[:, VW:], osb[:, VW:])

    nc.compile()
    return nc


def _feat(tab, x, pts):
    out = np.empty(pts.shape + (R,), dtype=np.float32)
    for t in range(R):
        out[..., t] = np.interp(pts, x, tab[:, t])
    return out


def _prep_in_maps(queries, key, value, W_k, W_q, W_v, Ls):
    import ml_dtypes

    bf16 = ml_dtypes.bfloat16
    f8 = ml_dtypes.float8_e4m3fn
    x, ftab, gtab = _basis()
    wv = W_v[0].astype(np.float32)

    # host projections (tiny, <1% of FLOPs — same as baseline)
    qf = np.einsum("hd,bqd->bqh", W_q, queries, optimize=True)
    kf = np.einsum("hd,bkd->bkh", W_k, key, optimize=True)

    def as_bf(a8):  # view fp8 bytes as bf16 carrier elements
        return a8.view(np.uint8).reshape(H, -1, 2).view(np.uint16).reshape(
            H, -1
        ).view(bf16)

    in_maps = []
    for b in range(B):
        L = int(Ls[b])
        # U[h, t*Q + q] = wv[h] * f_t(qf[b,q,h])
        fq = _feat(ftab, x, qf[b])                      # [Q, H, R]
        U = (fq * wv[None, :, None]).transpose(1, 2, 0)  # [H, R, Q]
        U = np.ascontiguousarray(U.reshape(H, R * Q))
        U_bf = U[:, : 2 * Q].astype(bf16)
        U_8 = U[:, 2 * Q :].astype(f8)

        # G[c, h, t*128 + j] = g_t(kf[b, c*128+j, h]), zero for k >= L;
        # pad chunks duplicate chunk 0 (finite scores under exp, V there is 0)
        gk = _feat(gtab, x, kf[b])                      # [K, H, R]
        gk[L:] = 0.0
        G = gk.transpose(1, 2, 0).reshape(H, R, NCH, 128)
        G = np.ascontiguousarray(G.transpose(2, 0, 1, 3)).reshape(
            NCH, H, R * 128
        )
        nreal = max(1, -(-L // 128))
        G[nreal:] = G[0]
        G_bf = G[:, :, : 2 * 128].astype(bf16)
        G_8 = G[:, :, 2 * 128 :].astype(f8)

        # V chunks with ones column; rows >= L zeroed
        Vv = np.zeros((K, VW), dtype=np.float32)
        Vv[:L, :V] = value[b, :L]
        Vv[:L, V] = 1.0
        Vv = Vv.reshape(NCH, 128, VW).astype(bf16)

        m = {
            "pk0": np.concatenate(
                [U_bf, G_bf[0], as_bf(U_8), as_bf(G_8[0])], axis=1
            ),
            "g123": np.concatenate(
                sum(([G_bf[c], as_bf(G_8[c])] for c in range(1, NCH)), []),
                axis=1,
            ),
            "pkv": np.ascontiguousarray(
                Vv.transpose(1, 0, 2).reshape(128, NCH * VW)
            ),
        }
        in_maps.append(m)
    return in_maps


def kernel(queries, key, value, W_k, W_q, W_v, valid_length):
    global _NC_CACHE, _LAST_RESULTS
    queries = np.asarray(queries, dtype=np.float32)
    key = np.asarray(key, dtype=np.float32)
    value = np.asarray(value, dtype=np.float32)
    W_k = np.asarray(W_k, dtype=np.float32)
    W_q = np.asarray(W_q, dtype=np.float32)
    W_v = np.asarray(W_v, dtype=np.float32)
    Ls = tuple(int(x) for x in np.asarray(valid_length).reshape(-1))
    assert len(Ls) == B and all(1 <= L <= K for L in Ls)

    if _NC_CACHE is None:
        _NC_CACHE = _build()
    nc = _NC_CACHE

    in_maps = _prep_in_maps(queries, key, value, W_k, W_q, W_v, Ls)

    from concourse.bass_utils import run_bass_kernel_spmd

    res = run_bass_kernel_spmd(nc, in_maps, core_ids=list(range(NCORES)))
    _LAST_RESULTS = res

    out = np.empty((B, Q, V), dtype=np.float32)
    for b in range(NCORES):
        raw = res.results[b]["out"].astype(np.float32).reshape(128, 2, VW)
        raw = raw.transpose(1, 0, 2).reshape(Q, VW)
        out[b] = raw[:, :V] / raw[:, V : V + 1]
    return out
